# revision 1
# baseline (speedup 1.0000x reference)
"""Trainium2 Bass kernel for nn_AttnCell (single-head attention with mask).

Full-problem shapes: inputs1 [4,4096,256] f32, inputs2 [4,4096,256] f32,
mask [4,4096,4096] i32, Wq/Wk/Wv [256,256] f32, bq/bk/bv [256] f32
-> out [4,4096,256] f32.

Sharding over 8 NeuronCores: core c handles batch b = c//2 and query-row half
h = c%2 (2048 query rows), with the full K/V rows for its batch replicated.
Each core runs an identical Bass program on its shard; the host concatenates
the 8 output shards.

Per-core algorithm (mathematically equal to the reference):
  q = x1 @ Wq + bq ; k = x2 @ Wk + bk ; v = x2 @ Wv + bv
  s = (q @ k.T) * (1/16)
  masked = s * mask           # == where(mask<=0, 1e-9, s) under exp, since
  p = exp(masked)             # exp(0) == exp(1e-9) == 1.0f exactly in fp32
  out = (p / p.sum(-1)) @ v   # no max-subtraction needed: |s| <= ~6
Scores are computed in float32r (tf32-like) matmuls; p and v are fp16 for the
second matmul; the softmax denominator comes from a ones-column appended to v
so it falls out of the same PSUM accumulation.
"""
from contextlib import ExitStack

import numpy as np

import concourse.bass as bass
import concourse.bacc as bacc
import concourse.tile as tile
import concourse.mybir as mybir
from concourse import masks
from concourse.bass_utils import run_bass_kernel_spmd

F32 = mybir.dt.float32
F32R = mybir.dt.float32r
F16 = mybir.dt.float16
I32 = mybir.dt.int32
U8 = mybir.dt.uint8

B = 4
N1 = 4096
N2 = 4096
D = 256
H = 256
N_CORES = 8
N1S = N1 // 2      # 2048 query rows per core
SCALE = 1.0 / 16.0  # 1/sqrt(H)

NT1 = N1S // 128   # 16 n1 tiles per core
NT2 = N2 // 128    # 32 n2 tiles
NB1 = N1S // 512   # 4 n1 blocks
NB2 = N2 // 512    # 8 n2 blocks


def _attn_body(tc, out, x1, x2, msk, wq, wk, wv, bq, bk, bv):
    nc = tc.nc
    Exp = mybir.ActivationFunctionType.Exp
    Copy = mybir.ActivationFunctionType.Copy
    Ident = mybir.ActivationFunctionType.Identity
    Mult = mybir.AluOpType.mult

    x1q = x1.ap().rearrange("(q a p) d -> q p a d", a=4, p=128)
    x2q = x2.ap().rearrange("(q a p) d -> q p a d", a=4, p=128)
    mskr = msk.ap().rearrange("(t p) m -> t p m", p=128)
    mskp = msk.ap().rearrange("(q i p) m -> q p i m", i=4, p=128)
    outr = out.ap().rearrange("(t p) h -> t p h", p=128)
    wqr = wq.ap().rearrange("(t p) h -> t p h", p=128)
    wkr = wk.ap().rearrange("(t p) h -> t p h", p=128)
    wvr = wv.ap().rearrange("(t p) h -> t p h", p=128)
    bqr = bq.ap().rearrange("(t p) -> t p", p=128)
    bkr = bk.ap().rearrange("(t p) -> t p", p=128)
    bvr = bv.ap()

    with ExitStack() as big_ctx:
        persist = big_ctx.enter_context(tc.tile_pool(name="persist", bufs=1))
        QT = persist.tile([128, 2, N1S], F32R)     # QT[p, ht, n1] = Q[n1, ht*128+p]
        KT = persist.tile([128, 2, N2], F32R)      # KT[p, ht, n2] = K[n2, ht*128+p]
        V = persist.tile([128, NT2, H + 2], F16)   # V[p, t2, :H]; col H = 1.0
        wsb = persist.tile([128, 3, 2, H], F32R)   # [p, {q,k,v}, dt, h]
        bsb = persist.tile([128, 2, 2], F32)       # [p, ht, {bq, bk}]
        bvsb = persist.tile([1, H], F32R)
        ones = persist.tile([1, 128], F32R)
        ident = persist.tile([128, 128], F32)

        masks.make_identity(nc, ident[:])
        xin_head = persist.tile([128, 2, 4, D], F32)
        nc.sync.dma_start(xin_head[:, 0], x1q[0])
        nc.sync.dma_start(xin_head[:, 1], x1q[1])
        nc.gpsimd.memset(V[:, :, H:H + 1], 1.0)
        wstage = persist.tile([128, 3, 2, H], F32)
        bvstage = persist.tile([1, H], F32)
        for w_i, w_ap in enumerate((wqr, wkr, wvr)):
            for dt_i in range(2):
                nc.sync.dma_start(wstage[:, w_i, dt_i, :], w_ap[dt_i])
                nc.vector.tensor_copy(
                    wsb[:, w_i, dt_i, :], wstage[:, w_i, dt_i, :])
        for ht in range(2):
            nc.sync.dma_start(bsb[:, ht, 0:1], bqr[ht])
            nc.sync.dma_start(bsb[:, ht, 1:2], bkr[ht])
        nc.sync.dma_start(bvstage[:], bvr)
        nc.vector.tensor_copy(bvsb[:], bvstage[:])
        onestage = persist.tile([1, 128], F32)
        nc.gpsimd.memset(onestage[:], 1.0)
        nc.vector.tensor_copy(ones[:], onestage[:])

        # ---- main-loop pools + early mask prefetch (overlaps preproc)
        mp = big_ctx.enter_context(tc.tile_pool(name="mask", bufs=2))
        pts, mpairs = {}, {}
        for pre_q in range(2):
            mpairs[pre_q] = mp.tile([128, 4, N2], U8, tag="mt",
                                    name=f"mt{pre_q}")
            nc.gpsimd.dma_start(mpairs[pre_q][:], mskp[pre_q])

        # ---- preprocessing: X^T via PE transpose, then QT/KT/V projections
        with ExitStack() as pre_ctx:
            pre = pre_ctx.enter_context(tc.tile_pool(name="pre", bufs=3))
            xt_pool = pre_ctx.enter_context(tc.tile_pool(name="xt", bufs=1))
            ps_t = pre_ctx.enter_context(
                tc.tile_pool(name="ps_t", bufs=2, space="PSUM"))
            ps_b = pre_ctx.enter_context(
                tc.tile_pool(name="ps_b", bufs=2, space="PSUM"))

            X1T = xt_pool.tile([128, 2, N1S], F32R)
            X2T = xt_pool.tile([128, 2, N2], F32R)

            for s_i, (srcq, ntiles, dst) in enumerate(
                    ((x1q, NT1 // 4, X1T), (x2q, NT2 // 4, X2T))):
                for tq in range(ntiles):
                    if s_i == 0 and tq < 2:
                        xin = xin_head[:, tq]
                    else:
                        xin = pre.tile([128, 4, D], F32, tag="xin",
                                       name=f"xin{s_i}_{tq}")
                        nc.sync.dma_start(xin[:], srcq[tq])
                    for a in range(4):
                        t = tq * 4 + a
                        for dt_i in range(2):
                            pt = ps_t.tile([128, 128], F32, tag="pt")
                            nc.tensor.matmul(
                                pt[:], xin[:, a, dt_i * 128:(dt_i + 1) * 128],
                                ident[:], is_transpose=True)
                            nc.vector.tensor_copy(
                                dst[:, dt_i, t * 128:(t + 1) * 128], pt[:])

            for w_i, XT, nblocks, dstT, b_col in (
                    (0, X1T, NB1, QT, 0), (1, X2T, NB2, KT, 1)):
                for j in range(nblocks):
                    for ht in range(2):
                        pq = ps_b.tile([128, 512], F32, tag="pq")
                        for dt_i in range(2):
                            nc.tensor.matmul(
                                pq[:],
                                wsb[:, w_i, dt_i,
                                    ht * 128:(ht + 1) * 128],
                                XT[:, dt_i,
                                   j * 512:(j + 1) * 512],
                                start=(dt_i == 0), stop=(dt_i == 1))
                        nc.scalar.activation(
                            dstT[:, ht, j * 512:(j + 1) * 512], pq[:],
                            Ident, bias=bsb[:, ht, b_col:b_col + 1], scale=1.0)

            for t2 in range(NT2):
                pv = ps_b.tile([128, 512], F32, tag="pq")
                nc.tensor.matmul(
                    pv[:, :H],
                    X2T[:, 0, t2 * 128:(t2 + 1) * 128],
                    wsb[:, 2, 0, :], start=True, stop=False)
                nc.tensor.matmul(
                    pv[:, :H],
                    X2T[:, 1, t2 * 128:(t2 + 1) * 128],
                    wsb[:, 2, 1, :], start=False, stop=False)
                nc.tensor.matmul(
                    pv[:, :H], ones[:], bvsb[:],
                    start=False, stop=True)
                nc.scalar.activation(V[:, t2, :H], pv[:, :H], Copy)

        # ---- main loop over n1 tiles
        pp = big_ctx.enter_context(tc.tile_pool(name="ptile", bufs=3))
        pt_pool = big_ctx.enter_context(tc.tile_pool(name="pt_pool", bufs=4))
        sp = big_ctx.enter_context(tc.tile_pool(name="small", bufs=3))
        ps_s = big_ctx.enter_context(
            tc.tile_pool(name="ps_s", bufs=3, space="PSUM"))
        ps_o = big_ctx.enter_context(
            tc.tile_pool(name="ps_o", bufs=2, space="PSUM"))
        obig = persist.tile([128, NT1, H], F32)

        # Software pipeline (depth 2): stage A(t1) = S matmuls + mask-mul +
        # exp + split xbar transposes; stage B(t1-2) = PV matmuls +
        # normalize. The 2-deep lag gives each xbar transpose a full
        # iteration of slack, so PE never stalls on it; mask prefetch is
        # emitted after the xbar so the transpose wins the DMA queue.
        LAG = 3
        for t1 in range(NT1 + LAG):
            have_b = t1 >= LAG
            if have_b:
                tb = t1 - LAG
                PTp = pts.pop(tb)
                o_ps = ps_o.tile([128, H + 1], F32, tag="o")

            def pv_group(g):
                # 8 of the 32 accumulating PV matmuls; interleaved between
                # S-blocks so PE fills s_ps slot-release stalls with PV work
                for t2 in range(g * 8, (g + 1) * 8):
                    nc.tensor.matmul(
                        o_ps[:], PTp[:, t2, :], V[:, t2, :H + 1],
                        start=(t2 == 0), stop=(t2 == NT2 - 1))

            if t1 < NT1:
                q, qi = t1 // 4, t1 % 4
                mtile = mpairs[q][:, qi]
                P = pp.tile([128, N2], F16, tag="p")
                PT = pt_pool.tile([128, NT2, 128], F16, tag="ptr")
                pts[t1] = PT
                for j in range(N2 // 1024):
                    s_ps = ps_s.tile([128, 1024], F32, tag="s")
                    for jj in range(2):
                        sl = s_ps[:, jj * 512:(jj + 1) * 512]
                        for ht in range(2):
                            nc.tensor.matmul(
                                sl,
                                QT[:, ht, t1 * 128:(t1 + 1) * 128],
                                KT[:, ht, j * 1024 + jj * 512:
                                   j * 1024 + (jj + 1) * 512],
                                start=(ht == 0), stop=(ht == 1))
                    masked = sp.tile([128, 1024], F32, tag="msk")
                    nc.vector.scalar_tensor_tensor(
                        out=masked[:], in0=s_ps[:], scalar=SCALE,
                        in1=mtile[:, j * 1024:(j + 1) * 1024],
                        op0=Mult, op1=Mult)
                    nc.scalar.activation(
                        P[:, j * 1024:(j + 1) * 1024], masked[:], Exp)
                    if j % 2 == 1:  # transpose each finished 2048 half
                        h = j // 2
                        nc.sync.dma_start(
                            PT[:, h * 16:(h + 1) * 16, :],
                            P[:, h * 2048:(h + 1) * 2048], transpose=True)
                    if have_b:
                        pv_group(j)
                if qi == 3:
                    mpairs.pop(q)
                    nq = q + 2
                    if nq < NT1 // 4:
                        nxt = mp.tile([128, 4, N2], U8, tag="mt",
                                      name=f"mtn{nq}")
                        mpairs[nq] = nxt
                        nc.gpsimd.dma_start(nxt[:], mskp[nq])
            if have_b:
                if t1 >= NT1:  # drain iterations: no A-stage to interleave
                    for g in range(4):
                        pv_group(g)
                zrec = sp.tile([128, 1], F32, tag="z")
                nc.vector.reciprocal(zrec[:], o_ps[:, H:H + 1])
                nc.scalar.activation(
                    obig[:, tb, :], o_ps[:, :H], Copy, scale=zrec[:])
        nc.sync.dma_start(
            out.ap().rearrange("(t p) h -> p t h", p=128), obig[:])


_NC_CACHE = None


def build_nc():
    global _NC_CACHE
    if _NC_CACHE is not None:
        return _NC_CACHE
    nc = bacc.Bacc("TRN2", target_bir_lowering=False, debug=False)
    x1 = nc.dram_tensor("x1", [N1S, D], F32, kind="ExternalInput")
    x2 = nc.dram_tensor("x2", [N2, D], F32, kind="ExternalInput")
    msk = nc.dram_tensor("msk", [N1S, N2], I32, kind="ExternalInput")
    wq = nc.dram_tensor("wq", [D, H], F32, kind="ExternalInput")
    wk = nc.dram_tensor("wk", [D, H], F32, kind="ExternalInput")
    wv = nc.dram_tensor("wv", [D, H], F32, kind="ExternalInput")
    bq = nc.dram_tensor("bq", [H], F32, kind="ExternalInput")
    bk = nc.dram_tensor("bk", [H], F32, kind="ExternalInput")
    bv = nc.dram_tensor("bv", [H], F32, kind="ExternalInput")
    out = nc.dram_tensor("out", [N1S, H], F32, kind="ExternalOutput")
    with tile.TileContext(nc) as tc:
        _attn_body(tc, out, x1, x2, msk, wq, wk, wv, bq, bk, bv)
    nc.compile()
    _NC_CACHE = nc
    return nc


def make_in_maps(inputs1, inputs2, mask, Wq, bq, Wk, bk, Wv, bv):
    inputs1 = np.ascontiguousarray(np.asarray(inputs1, dtype=np.float32))
    inputs2 = np.ascontiguousarray(np.asarray(inputs2, dtype=np.float32))
    mask = np.ascontiguousarray(np.asarray(mask, dtype=np.int32))
    com = {
        "wq": np.ascontiguousarray(np.asarray(Wq, dtype=np.float32)),
        "wk": np.ascontiguousarray(np.asarray(Wk, dtype=np.float32)),
        "wv": np.ascontiguousarray(np.asarray(Wv, dtype=np.float32)),
        "bq": np.ascontiguousarray(np.asarray(bq, dtype=np.float32)),
        "bk": np.ascontiguousarray(np.asarray(bk, dtype=np.float32)),
        "bv": np.ascontiguousarray(np.asarray(bv, dtype=np.float32)),
    }
    in_maps = []
    for c in range(N_CORES):
        b, half = c // 2, c % 2
        rows = slice(half * N1S, (half + 1) * N1S)
        in_maps.append({
            "x1": np.ascontiguousarray(inputs1[b, rows]),
            "x2": inputs2[b],
            "msk": np.ascontiguousarray(mask[b, rows]),
            **com,
        })
    return in_maps


def gather_out(results):
    out = np.empty((B, N1, H), np.float32)
    for c in range(N_CORES):
        b, half = c // 2, c % 2
        out[b, half * N1S:(half + 1) * N1S] = results[c]["out"]
    return out


def kernel(inputs1, inputs2, mask, Wq, bq, Wk, bk, Wv, bv):
    nc = build_nc()
    in_maps = make_in_maps(inputs1, inputs2, mask, Wq, bq, Wk, bk, Wv, bv)
    res = run_bass_kernel_spmd(nc, in_maps, list(range(N_CORES)))
    return gather_out(res.results)

